# revision 14
# baseline (speedup 1.0000x reference)
"""Multi-head causal attention (B=4, S=2048, D=1024, H=16) on 8 trn2 NeuronCores.

Sharding: core = (batch b, head-group g) with b in 0..3, g in 0..1.
Each core computes heads g*8..g*8+7 of batch b end-to-end (QKV projection,
causal softmax attention, and its partial slice of the output projection).
Host sums the two partial output projections per batch and adds bo.

On-device layout: activations are kept "transposed" ([channels, seq] with
channels on SBUF partitions) so every matmul streams a 512-wide free dim at
full fp32r rate:
  qT/kT:  [c, s]   c = head*64+dk channels of this core's 8 heads
  scoresT:[sk, sq] per head; head pairs (2t, 2t+1) are packed into row
          groups 0:64 / 64:128 of the PE array and adjacent PSUM banks
  vhx:    [sk, (h, 65)]  v-heads in natural [s, c] layout with a ones column
          appended per head, so attn@v also produces the softmax denominator
          (row 64 of the PSUM output) for free.
  outT:   [c, sq] normalized by 1/denominator, then Wo^T projection -> o[dout, s]

All matmul operands are float32r (TF32-like, ~1.5e-4 matmul rel err, full
PE rate at free dim >= 256). DRAM inputs are declared float32r so plain
HWDGE DMAs feed the PE directly (measured bit-identical to the explicit
rounding path).
"""

import sys

sys.path.insert(0, "/opt/trn_rl_repo")

import numpy as np

B, S, D, H, DK = 4, 2048, 1024, 16, 64
NCORES = 8
CPG = 512          # channels per core (8 heads)
HPC = 8            # heads per core
NB = 4             # sq blocks of 512
SQB = 512
NDT = D // 128     # 8 d-tiles
NCT = CPG // 128   # 4 c-tiles per core (= head pairs)
NST = S // 128     # 16 s-tiles

_PROGRAM = None


def build_program():
    import concourse.tile as tile
    from concourse import mybir, bacc

    F32 = mybir.dt.float32
    F32R = mybir.dt.float32r
    BF16 = mybir.dt.bfloat16
    AF = mybir.ActivationFunctionType
    ADD = mybir.AluOpType.add
    MUL = mybir.AluOpType.mult

    nc = bacc.Bacc("TRN2", target_bir_lowering=False, debug=False,
                   num_devices=NCORES)

    xq = nc.dram_tensor("xq", [D, S], BF16, kind="ExternalInput").ap()
    xk = nc.dram_tensor("xk", [D, S], BF16, kind="ExternalInput").ap()
    xv = nc.dram_tensor("xv", [D, S], BF16, kind="ExternalInput").ap()
    wq = nc.dram_tensor("wq", [D, CPG], BF16, kind="ExternalInput").ap()
    wk = nc.dram_tensor("wk", [D, CPG], BF16, kind="ExternalInput").ap()
    wv = nc.dram_tensor("wv", [D, CPG], BF16, kind="ExternalInput").ap()
    wo = nc.dram_tensor("wo", [CPG, D], F32R, kind="ExternalInput").ap()
    bqk = nc.dram_tensor("bqk", [128, 8], F32, kind="ExternalInput").ap()
    bv1 = nc.dram_tensor("bv1", [1, CPG], F32, kind="ExternalInput").ap()
    maskw = nc.dram_tensor("maskw", [128, 4, 1024], BF16,
                           kind="ExternalInput").ap()
    ones8 = nc.dram_tensor("ones8", [128, 8], F32R, kind="ExternalInput").ap()
    o = nc.dram_tensor("o", [D, S], F32, kind="ExternalOutput").ap()

    xq_r = xq.rearrange("(o p) s -> p o s", p=128)
    xk_r = xk.rearrange("(o p) s -> p o s", p=128)
    xv_r = xv.rearrange("(o p) s -> p o s", p=128)
    wq_r = wq.rearrange("(o p) c -> p o c", p=128)
    wk_r = wk.rearrange("(o p) c -> p o c", p=128)
    wv_r = wv.rearrange("(o p) c -> p o c", p=128)
    wo_r = wo.rearrange("(o p) c -> p o c", p=128)

    with tile.TileContext(nc) as tc:
        with (
            tc.tile_pool(name="wts", bufs=1) as wts,
            tc.tile_pool(name="kv", bufs=1) as kv,
            tc.tile_pool(name="stream", bufs=1) as strm,
            tc.tile_pool(name="phB2", bufs=2) as pB2,
            tc.tile_pool(name="phB3", bufs=3) as pB3,
            tc.tile_pool(name="ps_sc", bufs=2, space="PSUM") as ps_sc,
            tc.tile_pool(name="ps_pv", bufs=3, space="PSUM") as ps_pv,
            tc.tile_pool(name="ps_mm", bufs=1, space="PSUM") as ps_mm,
        ):
            # small constants first (cheap on the DMA queue)
            bqk_t = wts.tile([128, 8], F32)
            bv_row = wts.tile([1, CPG], F32)
            bvB_t = wts.tile([128, CPG], F32)
            ones_t = wts.tile([128, HPC], F32R)
            nc.sync.dma_start(out=bqk_t[:], in_=bqk[:])
            nc.sync.dma_start(out=bv_row[:], in_=bv1[:])
            nc.sync.dma_start(out=ones_t[:], in_=ones8[:])
            nc.gpsimd.partition_broadcast(bvB_t[:], bv_row[:])

            kT_t = kv.tile([128, NCT, S], F32R)
            vhx_t = kv.tile([128, NST, HPC, DK + 1], F32R)

            # ---- v projection (vhx) ----
            wv_t = strm.tile([128, NDT, CPG], BF16, tag="wkv")
            for c2 in range(2):
                nc.sync.dma_start(out=wv_t[:, 4 * c2:4 * c2 + 4, :],
                                  in_=wv_r[:, 4 * c2:4 * c2 + 4, :])
            xv_t = strm.tile([128, NDT, S], BF16, tag="xstream")
            for c4 in range(4):
                nc.sync.dma_start(out=xv_t[:, 2 * c4:2 * c4 + 2, :],
                                  in_=xv_r[:, 2 * c4:2 * c4 + 2, :])
            for st in range(NST):
                pv = ps_mm.tile([128, CPG], F32, tag="mm")
                for d in range(NDT):
                    nc.tensor.matmul(pv[:],
                                     xv_t[:, d, st * 128:(st + 1) * 128],
                                     wv_t[:, d, :],
                                     start=(d == 0), stop=(d == NDT - 1))
                nc.vector.tensor_tensor(
                    vhx_t[:, st, :, 0:DK],
                    pv.rearrange("p (h d) -> p h d", h=HPC),
                    bvB_t.rearrange("p (h d) -> p h d", h=HPC),
                    ADD)
                nc.vector.tensor_copy(vhx_t[:, st, :, DK:DK + 1],
                                      ones_t[:].unsqueeze(-1))

            # ---- q projection for block 0 (overlaps with k loads) ----
            wq_t = wts.tile([128, NDT, CPG], BF16)
            for c2 in range(2):
                nc.sync.dma_start(out=wq_t[:, 4 * c2:4 * c2 + 4, :],
                                  in_=wq_r[:, 4 * c2:4 * c2 + 4, :])

            def q_proj(blk):
                sq0 = blk * SQB
                xq_t = wts.tile([128, NDT, SQB], BF16, tag="xq")
                nc.sync.dma_start(out=xq_t[:], in_=xq_r[:, :, sq0:sq0 + SQB])
                qT_t = pB2.tile([128, NCT, SQB], F32R, tag="qT")
                for t in range(NCT):
                    pq = ps_mm.tile([128, SQB], F32, tag="mm")
                    for d in range(NDT):
                        nc.tensor.matmul(pq[:],
                                         wq_t[:, d, t * 128:(t + 1) * 128],
                                         xq_t[:, d, :],
                                         start=(d == 0), stop=(d == NDT - 1))
                    nc.vector.tensor_tensor(
                        qT_t[:, t, :], pq[:],
                        bqk_t[:, t:t + 1].to_broadcast((128, SQB)), ADD)
                return qT_t

            qT_blk = q_proj(0)

            # ---- k projection (kT) ----
            wk_t = strm.tile([128, NDT, CPG], BF16, tag="wkv")
            for c2 in range(2):
                nc.sync.dma_start(out=wk_t[:, 4 * c2:4 * c2 + 4, :],
                                  in_=wk_r[:, 4 * c2:4 * c2 + 4, :])
            xk_t = strm.tile([128, NDT, S], BF16, tag="xstream")
            for c4 in range(4):
                nc.sync.dma_start(out=xk_t[:, 2 * c4:2 * c4 + 2, :],
                                  in_=xk_r[:, 2 * c4:2 * c4 + 2, :])
            for t in range(NCT):
                for blk in range(NB):
                    pk = ps_mm.tile([128, SQB], F32, tag="mm")
                    for d in range(NDT):
                        nc.tensor.matmul(
                            pk[:],
                            wk_t[:, d, t * 128:(t + 1) * 128],
                            xk_t[:, d, blk * SQB:(blk + 1) * SQB],
                            start=(d == 0), stop=(d == NDT - 1))
                    nc.vector.tensor_tensor(
                        kT_t[:, t, blk * SQB:(blk + 1) * SQB], pk[:],
                        bqk_t[:, 4 + t:5 + t].to_broadcast((128, SQB)),
                        ADD)

            # remaining weights while k projection runs
            maskw_t = wts.tile([128, 4, 1024], BF16)
            nc.sync.dma_start(out=maskw_t[:], in_=maskw[:])
            wo_t = wts.tile([128, NCT, D], F32R)
            for c2 in range(2):
                nc.sync.dma_start(out=wo_t[:, 2 * c2:2 * c2 + 2, :],
                                  in_=wo_r[:, 2 * c2:2 * c2 + 2, :])

            def final_proj(outTn_t, sq0):
                for dt_i in range(8):
                    pf = ps_mm.tile([128, SQB], F32, tag="mm")
                    for t in range(NCT):
                        nc.tensor.matmul(
                            pf[:],
                            wo_t[:, t, dt_i * 128:(dt_i + 1) * 128],
                            outTn_t[:, t, :],
                            start=(t == 0), stop=(t == NCT - 1))
                    sf = pB2.tile([128, SQB], F32, tag="sf")
                    nc.vector.tensor_copy(sf[:], pf[:])
                    nc.sync.dma_start(
                        out=o[dt_i * 128:(dt_i + 1) * 128, sq0:sq0 + SQB],
                        in_=sf[:])

            pending = []

            # ---- per sq-block attention ----
            for blk in range(NB):
                sq0 = blk * SQB
                nsk = 4 * (blk + 1)
                qT_t = qT_blk
                outTn_t = pB2.tile([128, NCT, SQB], F32R, tag="outTn")
                for t in range(NCT):  # head pair (2t, 2t+1)
                    pvA = ps_pv.tile([65, SQB], F32, tag="pv")
                    pvB = ps_pv.tile([65, SQB], F32, tag="pv")
                    for j in range(nsk):
                        sk0 = j * 128
                        pg = ps_sc.tile([128, 1024], F32, tag="sc")
                        nc.tensor.matmul(pg[:, 0:SQB],
                                         kT_t[0:64, t, sk0:sk0 + 128],
                                         qT_t[0:64, t, :],
                                         start=True, stop=True)
                        nc.tensor.matmul(pg[:, SQB:2 * SQB],
                                         kT_t[64:128, t, sk0:sk0 + 128],
                                         qT_t[64:128, t, :],
                                         start=True, stop=True)
                        eg = pB2.tile([128, 1024], F32R, tag="eg")
                        nc.scalar.activation(eg[:], pg[:], AF.Exp,
                                             bias=0.0, scale=0.125)
                        if j >= blk * 4:
                            jo = j - blk * 4
                            nc.vector.tensor_tensor(
                                eg[:], eg[:], maskw_t[:, jo, :], MUL)
                        nc.tensor.matmul(pvA[:], vhx_t[:, j, 2 * t, :],
                                         eg[:, 0:SQB],
                                         start=(j == 0),
                                         stop=(j == nsk - 1))
                        nc.tensor.matmul(pvB[:], vhx_t[:, j, 2 * t + 1, :],
                                         eg[:, SQB:2 * SQB],
                                         start=(j == 0),
                                         stop=(j == nsk - 1))
                    for hp, pvh in ((0, pvA), (1, pvB)):
                        psl = slice(hp * 64, (hp + 1) * 64)
                        recip = pB2.tile([1, SQB], F32, tag="recip")
                        nc.vector.reciprocal(recip[:], pvh[64:65, :])
                        recipB = pB2.tile([64, SQB], F32, tag="recipB")
                        nc.gpsimd.partition_broadcast(recipB[:], recip[:])
                        nc.vector.tensor_tensor(outTn_t[psl, t, :],
                                                pvh[0:64, :], recipB[:],
                                                MUL)

                if blk + 1 < NB:
                    qT_blk = q_proj(blk + 1)
                pending.append((outTn_t, sq0))
                if blk >= 1:
                    final_proj(*pending.pop(0))
            while pending:
                final_proj(*pending.pop(0))

    nc.compile()
    return nc


def _get_program():
    global _PROGRAM
    if _PROGRAM is None:
        _PROGRAM = build_program()
    return _PROGRAM


def _make_maskw():
    import ml_dtypes
    p = np.arange(128, dtype=np.int64)[:, None]
    f = np.arange(512, dtype=np.int64)[None, :]
    mw = np.empty((128, 4, 1024), np.float32)
    for jo in range(4):
        m = (p <= (f - jo * 128)).astype(np.float32)
        mw[:, jo, 0:512] = m
        mw[:, jo, 512:1024] = m
    return mw.astype(ml_dtypes.bfloat16)


def make_in_maps(q, k, v, Wq, bq, Wk, bk, Wv, bv, Wo):
    import ml_dtypes
    BF = ml_dtypes.bfloat16
    wqT, wkT, wvT, woT = Wq.T, Wk.T, Wv.T, Wo.T
    mw = _make_maskw()
    xqT = [np.ascontiguousarray(q[b].T.astype(BF)) for b in range(B)]
    xkT = [np.ascontiguousarray(k[b].T.astype(BF)) for b in range(B)]
    xvT = [np.ascontiguousarray(v[b].T.astype(BF)) for b in range(B)]
    in_maps = []
    for core in range(NCORES):
        b, g = core // 2, core % 2
        cs = slice(g * CPG, (g + 1) * CPG)
        bqk_host = np.empty((128, 8), np.float32)
        for t in range(NCT):
            bqk_host[:, t] = bq[g * CPG + t * 128:g * CPG + (t + 1) * 128]
            bqk_host[:, 4 + t] = bk[g * CPG + t * 128:g * CPG + (t + 1) * 128]
        in_maps.append(dict(
            xq=xqT[b], xk=xkT[b], xv=xvT[b],
            wq=np.ascontiguousarray(wqT[:, cs].astype(BF)),
            wk=np.ascontiguousarray(wkT[:, cs].astype(BF)),
            wv=np.ascontiguousarray(wvT[:, cs].astype(BF)),
            wo=np.ascontiguousarray(woT[cs, :]),
            bqk=bqk_host,
            bv1=np.ascontiguousarray(bv[cs]).reshape(1, CPG),
            maskw=mw,
            ones8=np.ones((128, HPC), np.float32),
        ))
    return in_maps


def assemble_output(results, bo):
    out = np.empty((B, S, D), np.float32)
    for b in range(B):
        acc = results[2 * b]["o"] + results[2 * b + 1]["o"]  # [D, S]
        out[b] = acc.T + bo[None, :]
    return out


def _numpy_fallback(q, k, v, mask, Wq, bq, Wk, bk, Wv, bv, Wo, bo):
    def split_heads(x):
        return x.reshape(B, S, H, DK).transpose(0, 2, 1, 3)

    qh = split_heads(q @ Wq.T + bq)
    kh = split_heads(k @ Wk.T + bk)
    vh = split_heads(v @ Wv.T + bv)
    out = np.empty((B, H, S, DK), np.float32)
    m = np.broadcast_to(np.asarray(mask).reshape(-1, S, S)[-1], (S, S))
    for b in range(B):
        for h in range(H):
            s = (qh[b, h] @ kh[b, h].T) / np.float32(np.sqrt(DK))
            s = np.where(m == 0, np.float32(-1e9), s)
            s = s - s.max(axis=-1, keepdims=True)
            e = np.exp(s)
            a = e / e.sum(axis=-1, keepdims=True)
            out[b, h] = a @ vh[b, h]
    out = out.transpose(0, 2, 1, 3).reshape(B, S, D)
    return out @ Wo.T + bo


def kernel(q, k, v, mask, Wq, bq, Wk, bk, Wv, bv, Wo, bo):
    from concourse.bass_utils import run_bass_kernel_spmd

    q = np.ascontiguousarray(np.asarray(q), dtype=np.float32)
    k = np.ascontiguousarray(np.asarray(k), dtype=np.float32)
    v = np.ascontiguousarray(np.asarray(v), dtype=np.float32)
    Wq, Wk, Wv, Wo = (np.asarray(w, dtype=np.float32) for w in (Wq, Wk, Wv, Wo))
    bq, bk_, bv_, bo = (np.asarray(x, dtype=np.float32) for x in (bq, bk, bv, bo))

    mask_2d = np.asarray(mask).reshape(S, S)
    causal = bool(np.array_equal(mask_2d != 0, np.tril(np.ones((S, S), bool))))
    if not causal:
        return _numpy_fallback(q, k, v, mask, Wq, bq, Wk, bk_, Wv, bv_, Wo, bo)

    nc = _get_program()
    in_maps = make_in_maps(q, k, v, Wq, bq, Wk, bk_, Wv, bv_, Wo)
    res = run_bass_kernel_spmd(nc, in_maps, list(range(NCORES))).results
    return assemble_output(res, bo)


if __name__ == "__main__":
    nc = build_program()
    print("program built + compiled OK")


# revision 19
# speedup vs baseline: 1.0193x; 1.0193x over previous
"""Multi-head causal attention (B=4, S=2048, D=1024, H=16) on 8 trn2 NeuronCores.

Sharding: core = (batch b, head-group g) with b in 0..3, g in 0..1.
Each core computes heads g*8..g*8+7 of batch b end-to-end (QKV projection,
causal softmax attention, and its partial slice of the output projection).
Host sums the two partial output projections per batch and adds bo.

On-device layout: activations are kept "transposed" ([channels, seq] with
channels on SBUF partitions) so every matmul streams a 512-wide free dim at
full fp32r rate:
  qT/kT:  [c, s]   c = head*64+dk channels of this core's 8 heads
  scoresT:[sk, sq] per head; head pairs (2t, 2t+1) are packed into row
          groups 0:64 / 64:128 of the PE array and adjacent PSUM banks
  vhx:    [sk, (h, 65)]  v-heads in natural [s, c] layout with a ones column
          appended per head, so attn@v also produces the softmax denominator
          (row 64 of the PSUM output) for free.
  outT:   [c, sq] normalized by 1/denominator, then Wo^T projection -> o[dout, s]

Projection matmuls (Q/K/V) run in bf16 (halves the input DMA bytes); all
attention matmuls (scores, attn@v, Wo) run in float32r (TF32-like,
~1.5e-4 matmul rel err, full PE rate at free dim >= 256). DRAM inputs are
declared in the matmul dtype so plain HWDGE DMAs feed the PE directly.
Measured end-to-end max rel err vs the fp32 reference: ~2e-3.
"""

import sys

sys.path.insert(0, "/opt/trn_rl_repo")

import numpy as np

B, S, D, H, DK = 4, 2048, 1024, 16, 64
NCORES = 8
CPG = 512          # channels per core (8 heads)
HPC = 8            # heads per core
NB = 4             # sq blocks of 512
SQB = 512
NDT = D // 128     # 8 d-tiles
NCT = CPG // 128   # 4 c-tiles per core (= head pairs)
NST = S // 128     # 16 s-tiles

_PROGRAM = None


def build_program():
    import concourse.tile as tile
    from concourse import mybir, bacc

    F32 = mybir.dt.float32
    F32R = mybir.dt.float32r
    BF16 = mybir.dt.bfloat16
    AF = mybir.ActivationFunctionType
    ADD = mybir.AluOpType.add
    MUL = mybir.AluOpType.mult

    nc = bacc.Bacc("TRN2", target_bir_lowering=False, debug=False,
                   num_devices=NCORES)

    xq = nc.dram_tensor("xq", [D, S], BF16, kind="ExternalInput").ap()
    xk = nc.dram_tensor("xk", [D, S], BF16, kind="ExternalInput").ap()
    xv = nc.dram_tensor("xv", [D, S], BF16, kind="ExternalInput").ap()
    wq = nc.dram_tensor("wq", [D, CPG], BF16, kind="ExternalInput").ap()
    wk = nc.dram_tensor("wk", [D, CPG], BF16, kind="ExternalInput").ap()
    wv = nc.dram_tensor("wv", [D, CPG], BF16, kind="ExternalInput").ap()
    wo = nc.dram_tensor("wo", [CPG, D], F32R, kind="ExternalInput").ap()
    bqk = nc.dram_tensor("bqk", [128, 8], F32, kind="ExternalInput").ap()
    bv1 = nc.dram_tensor("bv1", [1, CPG], F32, kind="ExternalInput").ap()
    maskw = nc.dram_tensor("maskw", [128, 4, 1024], BF16,
                           kind="ExternalInput").ap()
    ones8 = nc.dram_tensor("ones8", [128, 8], F32R, kind="ExternalInput").ap()
    o = nc.dram_tensor("o", [D, S], F32, kind="ExternalOutput").ap()

    xq_r = xq.rearrange("(o p) s -> p o s", p=128)
    xk_r = xk.rearrange("(o p) s -> p o s", p=128)
    xv_r = xv.rearrange("(o p) s -> p o s", p=128)
    wq_r = wq.rearrange("(o p) c -> p o c", p=128)
    wk_r = wk.rearrange("(o p) c -> p o c", p=128)
    wv_r = wv.rearrange("(o p) c -> p o c", p=128)
    wo_r = wo.rearrange("(o p) c -> p o c", p=128)

    with tile.TileContext(nc) as tc:
        with (
            tc.tile_pool(name="wts", bufs=1) as wts,
            tc.tile_pool(name="kv", bufs=1) as kv,
            tc.tile_pool(name="stream", bufs=1) as strm,
            tc.tile_pool(name="phB2", bufs=2) as pB2,
            tc.tile_pool(name="phB3", bufs=3) as pB3,
            tc.tile_pool(name="ps_sc", bufs=2, space="PSUM") as ps_sc,
            tc.tile_pool(name="ps_pv", bufs=3, space="PSUM") as ps_pv,
            tc.tile_pool(name="ps_mm", bufs=1, space="PSUM") as ps_mm,
        ):
            # small constants first (cheap on the DMA queue)
            bqk_t = wts.tile([128, 8], F32)
            bv_row = wts.tile([1, CPG], F32)
            bvB_t = wts.tile([128, CPG], F32)
            ones_t = wts.tile([128, HPC], F32R)
            nc.sync.dma_start(out=bqk_t[:], in_=bqk[:])
            nc.sync.dma_start(out=bv_row[:], in_=bv1[:])
            nc.sync.dma_start(out=ones_t[:], in_=ones8[:])
            nc.gpsimd.partition_broadcast(bvB_t[:], bv_row[:])

            kT_t = kv.tile([128, NCT, S], F32R)
            vhx_t = kv.tile([128, NST, HPC, DK + 1], F32R)

            # ---- v projection (vhx) ----
            wv_t = strm.tile([128, NDT, CPG], BF16, tag="wkv")
            for c2 in range(2):
                nc.sync.dma_start(out=wv_t[:, 4 * c2:4 * c2 + 4, :],
                                  in_=wv_r[:, 4 * c2:4 * c2 + 4, :])
            xv_t = strm.tile([128, NDT, S], BF16, tag="xstream")
            for c4 in range(4):
                nc.sync.dma_start(out=xv_t[:, 2 * c4:2 * c4 + 2, :],
                                  in_=xv_r[:, 2 * c4:2 * c4 + 2, :])
            for st in range(NST):
                pv = ps_mm.tile([128, CPG], F32, tag="mm")
                for d in range(NDT):
                    nc.tensor.matmul(pv[:],
                                     xv_t[:, d, st * 128:(st + 1) * 128],
                                     wv_t[:, d, :],
                                     start=(d == 0), stop=(d == NDT - 1))
                nc.vector.tensor_tensor(
                    vhx_t[:, st, :, 0:DK],
                    pv.rearrange("p (h d) -> p h d", h=HPC),
                    bvB_t.rearrange("p (h d) -> p h d", h=HPC),
                    ADD)
                nc.vector.tensor_copy(vhx_t[:, st, :, DK:DK + 1],
                                      ones_t[:].unsqueeze(-1))

            # ---- q projection for block 0 (overlaps with k loads) ----
            wq_t = wts.tile([128, NDT, CPG], BF16)
            for c2 in range(2):
                nc.sync.dma_start(out=wq_t[:, 4 * c2:4 * c2 + 4, :],
                                  in_=wq_r[:, 4 * c2:4 * c2 + 4, :])

            def q_proj(blk):
                sq0 = blk * SQB
                xq_t = wts.tile([128, NDT, SQB], BF16, tag="xq")
                nc.sync.dma_start(out=xq_t[:], in_=xq_r[:, :, sq0:sq0 + SQB])
                qT_t = pB2.tile([128, NCT, SQB], F32R, tag="qT")
                for t in range(NCT):
                    pq = ps_mm.tile([128, SQB], F32, tag="mm")
                    for d in range(NDT):
                        nc.tensor.matmul(pq[:],
                                         wq_t[:, d, t * 128:(t + 1) * 128],
                                         xq_t[:, d, :],
                                         start=(d == 0), stop=(d == NDT - 1))
                    nc.vector.tensor_tensor(
                        qT_t[:, t, :], pq[:],
                        bqk_t[:, t:t + 1].to_broadcast((128, SQB)), ADD)
                return qT_t

            qT_blk = q_proj(0)

            # ---- k projection (kT) ----
            wk_t = strm.tile([128, NDT, CPG], BF16, tag="wkv")
            for c2 in range(2):
                nc.sync.dma_start(out=wk_t[:, 4 * c2:4 * c2 + 4, :],
                                  in_=wk_r[:, 4 * c2:4 * c2 + 4, :])
            xk_t = strm.tile([128, NDT, S], BF16, tag="xstream")
            for c4 in range(4):
                nc.sync.dma_start(out=xk_t[:, 2 * c4:2 * c4 + 2, :],
                                  in_=xk_r[:, 2 * c4:2 * c4 + 2, :])
            for t in range(NCT):
                for blk in range(NB):
                    pk = ps_mm.tile([128, SQB], F32, tag="mm")
                    for d in range(NDT):
                        nc.tensor.matmul(
                            pk[:],
                            wk_t[:, d, t * 128:(t + 1) * 128],
                            xk_t[:, d, blk * SQB:(blk + 1) * SQB],
                            start=(d == 0), stop=(d == NDT - 1))
                    nc.vector.tensor_tensor(
                        kT_t[:, t, blk * SQB:(blk + 1) * SQB], pk[:],
                        bqk_t[:, 4 + t:5 + t].to_broadcast((128, SQB)),
                        ADD)

            # remaining weights while k projection runs
            maskw_t = wts.tile([128, 4, 1024], BF16)
            nc.sync.dma_start(out=maskw_t[:], in_=maskw[:])
            wo_t = wts.tile([128, NCT, D], F32R)
            for c2 in range(2):
                nc.sync.dma_start(out=wo_t[:, 2 * c2:2 * c2 + 2, :],
                                  in_=wo_r[:, 2 * c2:2 * c2 + 2, :])

            def final_proj(outTn_t, sq0):
                for dt_i in range(8):
                    pf = ps_mm.tile([128, SQB], F32, tag="mm")
                    for t in range(NCT):
                        nc.tensor.matmul(
                            pf[:],
                            wo_t[:, t, dt_i * 128:(dt_i + 1) * 128],
                            outTn_t[:, t, :],
                            start=(t == 0), stop=(t == NCT - 1))
                    sf = pB2.tile([128, SQB], F32, tag="sf")
                    nc.vector.tensor_copy(sf[:], pf[:])
                    nc.sync.dma_start(
                        out=o[dt_i * 128:(dt_i + 1) * 128, sq0:sq0 + SQB],
                        in_=sf[:])

            pending = []

            # ---- per sq-block attention ----
            for blk in range(NB):
                sq0 = blk * SQB
                nsk = 4 * (blk + 1)
                qT_t = qT_blk
                outTn_t = pB2.tile([128, NCT, SQB], F32R, tag="outTn")
                for t in range(NCT):  # head pair (2t, 2t+1)
                    pvA = ps_pv.tile([65, SQB], F32, tag="pv")
                    pvB = ps_pv.tile([65, SQB], F32, tag="pv")
                    for j in range(nsk):
                        sk0 = j * 128
                        pg = ps_sc.tile([128, 1024], F32, tag="sc")
                        nc.tensor.matmul(pg[:, 0:SQB],
                                         kT_t[0:64, t, sk0:sk0 + 128],
                                         qT_t[0:64, t, :],
                                         start=True, stop=True)
                        nc.tensor.matmul(pg[:, SQB:2 * SQB],
                                         kT_t[64:128, t, sk0:sk0 + 128],
                                         qT_t[64:128, t, :],
                                         start=True, stop=True)
                        eg = pB2.tile([128, 1024], F32R, tag="eg")
                        nc.scalar.activation(eg[:], pg[:], AF.Exp,
                                             bias=0.0, scale=0.125)
                        if j >= blk * 4:
                            jo = j - blk * 4
                            nc.vector.tensor_tensor(
                                eg[:], eg[:], maskw_t[:, jo, :], MUL)
                        nc.tensor.matmul(pvA[:], vhx_t[:, j, 2 * t, :],
                                         eg[:, 0:SQB],
                                         start=(j == 0),
                                         stop=(j == nsk - 1))
                        nc.tensor.matmul(pvB[:], vhx_t[:, j, 2 * t + 1, :],
                                         eg[:, SQB:2 * SQB],
                                         start=(j == 0),
                                         stop=(j == nsk - 1))
                    for hp, pvh in ((0, pvA), (1, pvB)):
                        psl = slice(hp * 64, (hp + 1) * 64)
                        recip = pB2.tile([1, SQB], F32, tag="recip")
                        nc.vector.reciprocal(recip[:], pvh[64:65, :])
                        recipB = pB2.tile([64, SQB], F32, tag="recipB")
                        nc.gpsimd.partition_broadcast(recipB[:], recip[:])
                        nc.vector.tensor_tensor(outTn_t[psl, t, :],
                                                pvh[0:64, :], recipB[:],
                                                MUL)

                if blk + 1 < NB:
                    qT_blk = q_proj(blk + 1)
                pending.append((outTn_t, sq0))
                if blk >= 1:
                    final_proj(*pending.pop(0))
            while pending:
                final_proj(*pending.pop(0))

    nc.compile()
    return nc


def _get_program():
    global _PROGRAM
    if _PROGRAM is None:
        _PROGRAM = build_program()
    return _PROGRAM


def _make_maskw():
    import ml_dtypes
    p = np.arange(128, dtype=np.int64)[:, None]
    f = np.arange(512, dtype=np.int64)[None, :]
    mw = np.empty((128, 4, 1024), np.float32)
    for jo in range(4):
        m = (p <= (f - jo * 128)).astype(np.float32)
        mw[:, jo, 0:512] = m
        mw[:, jo, 512:1024] = m
    return mw.astype(ml_dtypes.bfloat16)


def make_in_maps(q, k, v, Wq, bq, Wk, bk, Wv, bv, Wo):
    import ml_dtypes
    BF = ml_dtypes.bfloat16
    wqT, wkT, wvT, woT = Wq.T, Wk.T, Wv.T, Wo.T
    mw = _make_maskw()
    xqT = [np.ascontiguousarray(q[b].T.astype(BF)) for b in range(B)]
    xkT = [np.ascontiguousarray(k[b].T.astype(BF)) for b in range(B)]
    xvT = [np.ascontiguousarray(v[b].T.astype(BF)) for b in range(B)]
    in_maps = []
    for core in range(NCORES):
        b, g = core // 2, core % 2
        cs = slice(g * CPG, (g + 1) * CPG)
        bqk_host = np.empty((128, 8), np.float32)
        for t in range(NCT):
            bqk_host[:, t] = bq[g * CPG + t * 128:g * CPG + (t + 1) * 128]
            bqk_host[:, 4 + t] = bk[g * CPG + t * 128:g * CPG + (t + 1) * 128]
        in_maps.append(dict(
            xq=xqT[b], xk=xkT[b], xv=xvT[b],
            wq=np.ascontiguousarray(wqT[:, cs].astype(BF)),
            wk=np.ascontiguousarray(wkT[:, cs].astype(BF)),
            wv=np.ascontiguousarray(wvT[:, cs].astype(BF)),
            wo=np.ascontiguousarray(woT[cs, :]),
            bqk=bqk_host,
            bv1=np.ascontiguousarray(bv[cs]).reshape(1, CPG),
            maskw=mw,
            ones8=np.ones((128, HPC), np.float32),
        ))
    return in_maps


def assemble_output(results, bo):
    out = np.empty((B, S, D), np.float32)
    for b in range(B):
        acc = results[2 * b]["o"] + results[2 * b + 1]["o"]  # [D, S]
        out[b] = acc.T + bo[None, :]
    return out


def _numpy_fallback(q, k, v, mask, Wq, bq, Wk, bk, Wv, bv, Wo, bo):
    def split_heads(x):
        return x.reshape(B, S, H, DK).transpose(0, 2, 1, 3)

    qh = split_heads(q @ Wq.T + bq)
    kh = split_heads(k @ Wk.T + bk)
    vh = split_heads(v @ Wv.T + bv)
    out = np.empty((B, H, S, DK), np.float32)
    m = np.broadcast_to(np.asarray(mask).reshape(-1, S, S)[-1], (S, S))
    for b in range(B):
        for h in range(H):
            s = (qh[b, h] @ kh[b, h].T) / np.float32(np.sqrt(DK))
            s = np.where(m == 0, np.float32(-1e9), s)
            s = s - s.max(axis=-1, keepdims=True)
            e = np.exp(s)
            a = e / e.sum(axis=-1, keepdims=True)
            out[b, h] = a @ vh[b, h]
    out = out.transpose(0, 2, 1, 3).reshape(B, S, D)
    return out @ Wo.T + bo


def kernel(q, k, v, mask, Wq, bq, Wk, bk, Wv, bv, Wo, bo):
    from concourse.bass_utils import run_bass_kernel_spmd

    q = np.ascontiguousarray(np.asarray(q), dtype=np.float32)
    k = np.ascontiguousarray(np.asarray(k), dtype=np.float32)
    v = np.ascontiguousarray(np.asarray(v), dtype=np.float32)
    Wq, Wk, Wv, Wo = (np.asarray(w, dtype=np.float32) for w in (Wq, Wk, Wv, Wo))
    bq, bk_, bv_, bo = (np.asarray(x, dtype=np.float32) for x in (bq, bk, bv, bo))

    mask_2d = np.asarray(mask).reshape(S, S)
    causal = bool(np.array_equal(mask_2d != 0, np.tril(np.ones((S, S), bool))))
    if not causal:
        return _numpy_fallback(q, k, v, mask, Wq, bq, Wk, bk_, Wv, bv_, Wo, bo)

    nc = _get_program()
    in_maps = make_in_maps(q, k, v, Wq, bq, Wk, bk_, Wv, bv_, Wo)
    res = run_bass_kernel_spmd(nc, in_maps, list(range(NCORES))).results
    return assemble_output(res, bo)


if __name__ == "__main__":
    nc = build_program()
    print("program built + compiled OK")


# revision 20
# speedup vs baseline: 1.0611x; 1.0410x over previous
"""Multi-head causal attention (B=4, S=2048, D=1024, H=16) on 8 trn2 NeuronCores.

Sharding: core = (batch b, head-group g) with b in 0..3, g in 0..1.
Each core computes heads g*8..g*8+7 of batch b end-to-end (QKV projection,
causal softmax attention, and its partial slice of the output projection).
Host sums the two partial output projections per batch and adds bo.

On-device layout: activations are kept "transposed" ([channels, seq] with
channels on SBUF partitions) so every matmul streams a 512-wide free dim at
full fp32r rate:
  qT/kT:  [c, s]   c = head*64+dk channels of this core's 8 heads
  scoresT:[sk, sq] per head; head pairs (2t, 2t+1) are packed into row
          groups 0:64 / 64:128 of the PE array and adjacent PSUM banks
  vhx:    [sk, (h, 65)]  v-heads in natural [s, c] layout with a ones column
          appended per head, so attn@v also produces the softmax denominator
          (row 64 of the PSUM output) for free.
  outT:   [c, sq] normalized by 1/denominator, then Wo^T projection -> o[dout, s]

Projection matmuls (Q/K/V) run in bf16 (halves the input DMA bytes); all
attention matmuls (scores, attn@v, Wo) run in float32r (TF32-like,
~1.5e-4 matmul rel err, full PE rate at free dim >= 256). DRAM inputs are
declared in the matmul dtype so plain HWDGE DMAs feed the PE directly.
Measured end-to-end max rel err vs the fp32 reference: ~2e-3.
"""

import sys

sys.path.insert(0, "/opt/trn_rl_repo")

import numpy as np

B, S, D, H, DK = 4, 2048, 1024, 16, 64
NCORES = 8
CPG = 512          # channels per core (8 heads)
HPC = 8            # heads per core
NB = 4             # sq blocks of 512
SQB = 512
NDT = D // 128     # 8 d-tiles
NCT = CPG // 128   # 4 c-tiles per core (= head pairs)
NST = S // 128     # 16 s-tiles

_PROGRAM = None


def build_program():
    import concourse.tile as tile
    from concourse import mybir, bacc

    F32 = mybir.dt.float32
    F32R = mybir.dt.float32r
    BF16 = mybir.dt.bfloat16
    AF = mybir.ActivationFunctionType
    ADD = mybir.AluOpType.add
    MUL = mybir.AluOpType.mult

    nc = bacc.Bacc("TRN2", target_bir_lowering=False, debug=False,
                   num_devices=NCORES)

    xq = nc.dram_tensor("xq", [D, S], BF16, kind="ExternalInput").ap()
    xk = nc.dram_tensor("xk", [D, S], BF16, kind="ExternalInput").ap()
    xv = nc.dram_tensor("xv", [D, S], BF16, kind="ExternalInput").ap()
    wq = nc.dram_tensor("wq", [D, CPG], BF16, kind="ExternalInput").ap()
    wk = nc.dram_tensor("wk", [D, CPG], BF16, kind="ExternalInput").ap()
    wv = nc.dram_tensor("wv", [D, CPG], BF16, kind="ExternalInput").ap()
    wo = nc.dram_tensor("wo", [CPG, D], F32R, kind="ExternalInput").ap()
    bqk = nc.dram_tensor("bqk", [128, 8], F32, kind="ExternalInput").ap()
    bv1 = nc.dram_tensor("bv1", [1, CPG], F32, kind="ExternalInput").ap()
    maskw = nc.dram_tensor("maskw", [128, 4, 1024], BF16,
                           kind="ExternalInput").ap()
    ones8 = nc.dram_tensor("ones8", [128, 8], F32R, kind="ExternalInput").ap()
    o = nc.dram_tensor("o", [D, S], F32, kind="ExternalOutput").ap()

    xq_r = xq.rearrange("(o p) s -> p o s", p=128)
    xk_r = xk.rearrange("(o p) s -> p o s", p=128)
    xv_r = xv.rearrange("(o p) s -> p o s", p=128)
    wq_r = wq.rearrange("(o p) c -> p o c", p=128)
    wk_r = wk.rearrange("(o p) c -> p o c", p=128)
    wv_r = wv.rearrange("(o p) c -> p o c", p=128)
    wo_r = wo.rearrange("(o p) c -> p o c", p=128)

    with tile.TileContext(nc) as tc:
        with (
            tc.tile_pool(name="wts", bufs=1) as wts,
            tc.tile_pool(name="kv", bufs=1) as kv,
            tc.tile_pool(name="stream", bufs=1) as strm,
            tc.tile_pool(name="phB2", bufs=2) as pB2,
            tc.tile_pool(name="phB3", bufs=3) as pB3,
            tc.tile_pool(name="ps_sc", bufs=2, space="PSUM") as ps_sc,
            tc.tile_pool(name="ps_pv", bufs=3, space="PSUM") as ps_pv,
            tc.tile_pool(name="ps_mm", bufs=1, space="PSUM") as ps_mm,
        ):
            # small constants first (cheap on the DMA queue)
            bqk_t = wts.tile([128, 8], F32)
            bv_row = wts.tile([1, CPG], F32)
            bvB_t = wts.tile([128, CPG], F32)
            ones_t = wts.tile([128, HPC], F32R)
            nc.sync.dma_start(out=bqk_t[:], in_=bqk[:])
            nc.sync.dma_start(out=bv_row[:], in_=bv1[:])
            nc.sync.dma_start(out=ones_t[:], in_=ones8[:])
            nc.gpsimd.partition_broadcast(bvB_t[:], bv_row[:])

            kT_t = kv.tile([128, NCT, S], F32R)
            vhx_t = kv.tile([128, NST, HPC, DK + 1], F32R)

            # ---- v projection (vhx) ----
            wv_t = strm.tile([128, NDT, CPG], BF16, tag="wkv")
            for c2 in range(2):
                nc.sync.dma_start(out=wv_t[:, 4 * c2:4 * c2 + 4, :],
                                  in_=wv_r[:, 4 * c2:4 * c2 + 4, :])
            xv_t = strm.tile([128, NDT, S], BF16, tag="xstream")
            for c4 in range(4):
                nc.sync.dma_start(out=xv_t[:, 2 * c4:2 * c4 + 2, :],
                                  in_=xv_r[:, 2 * c4:2 * c4 + 2, :])
            for st in range(NST):
                pv = ps_mm.tile([128, CPG], F32, tag="mm")
                for d in range(NDT):
                    nc.tensor.matmul(pv[:],
                                     xv_t[:, d, st * 128:(st + 1) * 128],
                                     wv_t[:, d, :],
                                     start=(d == 0), stop=(d == NDT - 1))
                nc.vector.tensor_tensor(
                    vhx_t[:, st, :, 0:DK],
                    pv.rearrange("p (h d) -> p h d", h=HPC),
                    bvB_t.rearrange("p (h d) -> p h d", h=HPC),
                    ADD)
                nc.vector.tensor_copy(vhx_t[:, st, :, DK:DK + 1],
                                      ones_t[:].unsqueeze(-1))

            # ---- q projection for block 0 (overlaps with k loads) ----
            wq_t = wts.tile([128, NDT, CPG], BF16)
            for c2 in range(2):
                nc.sync.dma_start(out=wq_t[:, 4 * c2:4 * c2 + 4, :],
                                  in_=wq_r[:, 4 * c2:4 * c2 + 4, :])

            def q_proj(blk):
                sq0 = blk * SQB
                xq_t = wts.tile([128, NDT, SQB], BF16, tag="xq")
                nc.sync.dma_start(out=xq_t[:], in_=xq_r[:, :, sq0:sq0 + SQB])
                qT_t = pB2.tile([128, NCT, SQB], F32R, tag="qT")
                for t in range(NCT):
                    pq = ps_mm.tile([128, SQB], F32, tag="mm")
                    for d in range(NDT):
                        nc.tensor.matmul(pq[:],
                                         wq_t[:, d, t * 128:(t + 1) * 128],
                                         xq_t[:, d, :],
                                         start=(d == 0), stop=(d == NDT - 1))
                    nc.vector.tensor_tensor(
                        qT_t[:, t, :], pq[:],
                        bqk_t[:, t:t + 1].to_broadcast((128, SQB)), ADD)
                return qT_t

            qT_blk = q_proj(0)

            # ---- k projection (kT) ----
            wk_t = strm.tile([128, NDT, CPG], BF16, tag="wkv")
            for c2 in range(2):
                nc.sync.dma_start(out=wk_t[:, 4 * c2:4 * c2 + 4, :],
                                  in_=wk_r[:, 4 * c2:4 * c2 + 4, :])
            xk_t = strm.tile([128, NDT, S], BF16, tag="xstream")
            for c4 in range(4):
                nc.sync.dma_start(out=xk_t[:, 2 * c4:2 * c4 + 2, :],
                                  in_=xk_r[:, 2 * c4:2 * c4 + 2, :])
            for t in range(NCT):
                for blk in range(NB):
                    pk = ps_mm.tile([128, SQB], F32, tag="mm")
                    for d in range(NDT):
                        nc.tensor.matmul(
                            pk[:],
                            wk_t[:, d, t * 128:(t + 1) * 128],
                            xk_t[:, d, blk * SQB:(blk + 1) * SQB],
                            start=(d == 0), stop=(d == NDT - 1))
                    nc.vector.tensor_tensor(
                        kT_t[:, t, blk * SQB:(blk + 1) * SQB], pk[:],
                        bqk_t[:, 4 + t:5 + t].to_broadcast((128, SQB)),
                        ADD)

            # remaining weights while k projection runs
            maskw_t = wts.tile([128, 4, 1024], BF16)
            nc.sync.dma_start(out=maskw_t[:], in_=maskw[:])
            wo_t = wts.tile([128, NCT, D], F32R)
            for c2 in range(2):
                nc.sync.dma_start(out=wo_t[:, 2 * c2:2 * c2 + 2, :],
                                  in_=wo_r[:, 2 * c2:2 * c2 + 2, :])

            def final_proj(outTn_t, sq0):
                for dt_i in range(8):
                    pf = ps_mm.tile([128, SQB], F32, tag="mm")
                    for t in range(NCT):
                        nc.tensor.matmul(
                            pf[:],
                            wo_t[:, t, dt_i * 128:(dt_i + 1) * 128],
                            outTn_t[:, t, :],
                            start=(t == 0), stop=(t == NCT - 1))
                    sf = pB2.tile([128, SQB], F32, tag="sf")
                    nc.vector.tensor_copy(sf[:], pf[:])
                    nc.sync.dma_start(
                        out=o[dt_i * 128:(dt_i + 1) * 128, sq0:sq0 + SQB],
                        in_=sf[:])

            pending = []

            # ---- per sq-block attention ----
            for blk in range(NB):
                sq0 = blk * SQB
                nsk = 4 * (blk + 1)
                qT_t = qT_blk
                outTn_t = pB2.tile([128, NCT, SQB], F32R, tag="outTn")
                for t in range(NCT):  # head pair (2t, 2t+1)
                    pvA = ps_pv.tile([65, SQB], F32, tag="pv")
                    pvB = ps_pv.tile([65, SQB], F32, tag="pv")
                    for j in range(nsk):
                        sk0 = j * 128
                        pg = ps_sc.tile([128, 1024], F32, tag="sc")
                        nc.tensor.matmul(pg[:, 0:SQB],
                                         kT_t[0:64, t, sk0:sk0 + 128],
                                         qT_t[0:64, t, :],
                                         start=True, stop=True)
                        nc.tensor.matmul(pg[:, SQB:2 * SQB],
                                         kT_t[64:128, t, sk0:sk0 + 128],
                                         qT_t[64:128, t, :],
                                         start=True, stop=True)
                        eg = pB3.tile([128, 1024], F32R, tag="eg")
                        nc.scalar.activation(eg[:], pg[:], AF.Exp,
                                             bias=0.0, scale=0.125)
                        if j >= blk * 4:
                            jo = j - blk * 4
                            nc.vector.tensor_tensor(
                                eg[:], eg[:], maskw_t[:, jo, :], MUL)
                        nc.tensor.matmul(pvA[:], vhx_t[:, j, 2 * t, :],
                                         eg[:, 0:SQB],
                                         start=(j == 0),
                                         stop=(j == nsk - 1))
                        nc.tensor.matmul(pvB[:], vhx_t[:, j, 2 * t + 1, :],
                                         eg[:, SQB:2 * SQB],
                                         start=(j == 0),
                                         stop=(j == nsk - 1))
                    for hp, pvh in ((0, pvA), (1, pvB)):
                        psl = slice(hp * 64, (hp + 1) * 64)
                        recip = pB2.tile([1, SQB], F32, tag="recip")
                        nc.vector.reciprocal(recip[:], pvh[64:65, :])
                        recipB = pB2.tile([64, SQB], F32, tag="recipB")
                        nc.gpsimd.partition_broadcast(recipB[:], recip[:])
                        nc.vector.tensor_tensor(outTn_t[psl, t, :],
                                                pvh[0:64, :], recipB[:],
                                                MUL)

                if blk + 1 < NB:
                    qT_blk = q_proj(blk + 1)
                pending.append((outTn_t, sq0))
                if blk >= 1:
                    final_proj(*pending.pop(0))
            while pending:
                final_proj(*pending.pop(0))

    nc.compile()
    return nc


def _get_program():
    global _PROGRAM
    if _PROGRAM is None:
        _PROGRAM = build_program()
    return _PROGRAM


def _make_maskw():
    import ml_dtypes
    p = np.arange(128, dtype=np.int64)[:, None]
    f = np.arange(512, dtype=np.int64)[None, :]
    mw = np.empty((128, 4, 1024), np.float32)
    for jo in range(4):
        m = (p <= (f - jo * 128)).astype(np.float32)
        mw[:, jo, 0:512] = m
        mw[:, jo, 512:1024] = m
    return mw.astype(ml_dtypes.bfloat16)


def make_in_maps(q, k, v, Wq, bq, Wk, bk, Wv, bv, Wo):
    import ml_dtypes
    BF = ml_dtypes.bfloat16
    wqT, wkT, wvT, woT = Wq.T, Wk.T, Wv.T, Wo.T
    mw = _make_maskw()
    xqT = [np.ascontiguousarray(q[b].T.astype(BF)) for b in range(B)]
    xkT = [np.ascontiguousarray(k[b].T.astype(BF)) for b in range(B)]
    xvT = [np.ascontiguousarray(v[b].T.astype(BF)) for b in range(B)]
    in_maps = []
    for core in range(NCORES):
        b, g = core // 2, core % 2
        cs = slice(g * CPG, (g + 1) * CPG)
        bqk_host = np.empty((128, 8), np.float32)
        for t in range(NCT):
            bqk_host[:, t] = bq[g * CPG + t * 128:g * CPG + (t + 1) * 128]
            bqk_host[:, 4 + t] = bk[g * CPG + t * 128:g * CPG + (t + 1) * 128]
        in_maps.append(dict(
            xq=xqT[b], xk=xkT[b], xv=xvT[b],
            wq=np.ascontiguousarray(wqT[:, cs].astype(BF)),
            wk=np.ascontiguousarray(wkT[:, cs].astype(BF)),
            wv=np.ascontiguousarray(wvT[:, cs].astype(BF)),
            wo=np.ascontiguousarray(woT[cs, :]),
            bqk=bqk_host,
            bv1=np.ascontiguousarray(bv[cs]).reshape(1, CPG),
            maskw=mw,
            ones8=np.ones((128, HPC), np.float32),
        ))
    return in_maps


def assemble_output(results, bo):
    out = np.empty((B, S, D), np.float32)
    for b in range(B):
        acc = results[2 * b]["o"] + results[2 * b + 1]["o"]  # [D, S]
        out[b] = acc.T + bo[None, :]
    return out


def _numpy_fallback(q, k, v, mask, Wq, bq, Wk, bk, Wv, bv, Wo, bo):
    def split_heads(x):
        return x.reshape(B, S, H, DK).transpose(0, 2, 1, 3)

    qh = split_heads(q @ Wq.T + bq)
    kh = split_heads(k @ Wk.T + bk)
    vh = split_heads(v @ Wv.T + bv)
    out = np.empty((B, H, S, DK), np.float32)
    m = np.broadcast_to(np.asarray(mask).reshape(-1, S, S)[-1], (S, S))
    for b in range(B):
        for h in range(H):
            s = (qh[b, h] @ kh[b, h].T) / np.float32(np.sqrt(DK))
            s = np.where(m == 0, np.float32(-1e9), s)
            s = s - s.max(axis=-1, keepdims=True)
            e = np.exp(s)
            a = e / e.sum(axis=-1, keepdims=True)
            out[b, h] = a @ vh[b, h]
    out = out.transpose(0, 2, 1, 3).reshape(B, S, D)
    return out @ Wo.T + bo


def kernel(q, k, v, mask, Wq, bq, Wk, bk, Wv, bv, Wo, bo):
    from concourse.bass_utils import run_bass_kernel_spmd

    q = np.ascontiguousarray(np.asarray(q), dtype=np.float32)
    k = np.ascontiguousarray(np.asarray(k), dtype=np.float32)
    v = np.ascontiguousarray(np.asarray(v), dtype=np.float32)
    Wq, Wk, Wv, Wo = (np.asarray(w, dtype=np.float32) for w in (Wq, Wk, Wv, Wo))
    bq, bk_, bv_, bo = (np.asarray(x, dtype=np.float32) for x in (bq, bk, bv, bo))

    mask_2d = np.asarray(mask).reshape(S, S)
    causal = bool(np.array_equal(mask_2d != 0, np.tril(np.ones((S, S), bool))))
    if not causal:
        return _numpy_fallback(q, k, v, mask, Wq, bq, Wk, bk_, Wv, bv_, Wo, bo)

    nc = _get_program()
    in_maps = make_in_maps(q, k, v, Wq, bq, Wk, bk_, Wv, bv_, Wo)
    res = run_bass_kernel_spmd(nc, in_maps, list(range(NCORES))).results
    return assemble_output(res, bo)


if __name__ == "__main__":
    nc = build_program()
    print("program built + compiled OK")


# revision 22
# speedup vs baseline: 1.1064x; 1.0427x over previous
"""Multi-head causal attention (B=4, S=2048, D=1024, H=16) on 8 trn2 NeuronCores.

Sharding: core = (batch b, head-group g) with b in 0..3, g in 0..1.
Each core computes heads g*8..g*8+7 of batch b end-to-end (QKV projection,
causal softmax attention, and its partial slice of the output projection).
Host sums the two partial output projections per batch and adds bo.

On-device layout: activations are kept "transposed" ([channels, seq] with
channels on SBUF partitions) so every matmul streams a 512-wide free dim at
full fp32r rate:
  qT/kT:  [c, s]   c = head*64+dk channels of this core's 8 heads
  scoresT:[sk, sq] per head; head pairs (2t, 2t+1) are packed into row
          groups 0:64 / 64:128 of the PE array and adjacent PSUM banks
  vhx:    [sk, (h, 65)]  v-heads in natural [s, c] layout with a ones column
          appended per head, so attn@v also produces the softmax denominator
          (row 64 of the PSUM output) for free.
  outT:   [c, sq] normalized by 1/denominator, then Wo^T projection -> o[dout, s]

Projection matmuls (Q/K/V) run in bf16 (halves the input DMA bytes); all
attention matmuls (scores, attn@v, Wo) run in float32r (TF32-like,
~1.5e-4 matmul rel err, full PE rate at free dim >= 256). DRAM inputs are
declared in the matmul dtype so plain HWDGE DMAs feed the PE directly.
Measured end-to-end max rel err vs the fp32 reference: ~2e-3.
"""

import sys

sys.path.insert(0, "/opt/trn_rl_repo")

import numpy as np

B, S, D, H, DK = 4, 2048, 1024, 16, 64
NCORES = 8
CPG = 512          # channels per core (8 heads)
HPC = 8            # heads per core
NB = 4             # sq blocks of 512
SQB = 512
NDT = D // 128     # 8 d-tiles
NCT = CPG // 128   # 4 c-tiles per core (= head pairs)
NST = S // 128     # 16 s-tiles

_PROGRAM = None


def build_program():
    import concourse.tile as tile
    from concourse import mybir, bacc

    F32 = mybir.dt.float32
    F32R = mybir.dt.float32r
    BF16 = mybir.dt.bfloat16
    AF = mybir.ActivationFunctionType
    ADD = mybir.AluOpType.add
    MUL = mybir.AluOpType.mult

    nc = bacc.Bacc("TRN2", target_bir_lowering=False, debug=False,
                   num_devices=NCORES)

    xq = nc.dram_tensor("xq", [D, S], BF16, kind="ExternalInput").ap()
    xk = nc.dram_tensor("xk", [D, S], BF16, kind="ExternalInput").ap()
    xv = nc.dram_tensor("xv", [D, S], BF16, kind="ExternalInput").ap()
    wq = nc.dram_tensor("wq", [D, CPG], BF16, kind="ExternalInput").ap()
    wk = nc.dram_tensor("wk", [D, CPG], BF16, kind="ExternalInput").ap()
    wv = nc.dram_tensor("wv", [D, CPG], BF16, kind="ExternalInput").ap()
    wo = nc.dram_tensor("wo", [CPG, D], F32R, kind="ExternalInput").ap()
    bqk = nc.dram_tensor("bqk", [128, 8], F32, kind="ExternalInput").ap()
    bv1 = nc.dram_tensor("bv1", [1, CPG], F32, kind="ExternalInput").ap()
    maskw = nc.dram_tensor("maskw", [128, 4, 1024], BF16,
                           kind="ExternalInput").ap()
    ones8 = nc.dram_tensor("ones8", [128, 8], F32R, kind="ExternalInput").ap()
    o = nc.dram_tensor("o", [D, S], F32, kind="ExternalOutput").ap()

    xq_r = xq.rearrange("(o p) s -> p o s", p=128)
    xk_r = xk.rearrange("(o p) s -> p o s", p=128)
    xv_r = xv.rearrange("(o p) s -> p o s", p=128)
    wq_r = wq.rearrange("(o p) c -> p o c", p=128)
    wk_r = wk.rearrange("(o p) c -> p o c", p=128)
    wv_r = wv.rearrange("(o p) c -> p o c", p=128)
    wo_r = wo.rearrange("(o p) c -> p o c", p=128)

    with tile.TileContext(nc) as tc:
        with (
            tc.tile_pool(name="wts", bufs=1) as wts,
            tc.tile_pool(name="kv", bufs=1) as kv,
            tc.tile_pool(name="stream", bufs=1) as strm,
            tc.tile_pool(name="phB2", bufs=2) as pB2,
            tc.tile_pool(name="phB3", bufs=3) as pB3,
            tc.tile_pool(name="ps_sc", bufs=2, space="PSUM") as ps_sc,
            tc.tile_pool(name="ps_pv", bufs=3, space="PSUM") as ps_pv,
            tc.tile_pool(name="ps_mm", bufs=1, space="PSUM") as ps_mm,
        ):
            # small constants first (cheap on the DMA queue)
            bqk_t = wts.tile([128, 8], F32)
            bv_row = wts.tile([1, CPG], F32)
            bvB_t = wts.tile([128, CPG], F32)
            ones_t = wts.tile([128, HPC], F32R)
            nc.sync.dma_start(out=bqk_t[:], in_=bqk[:])
            nc.sync.dma_start(out=bv_row[:], in_=bv1[:])
            nc.sync.dma_start(out=ones_t[:], in_=ones8[:])
            nc.gpsimd.partition_broadcast(bvB_t[:], bv_row[:])

            kT_t = kv.tile([128, NCT, S], F32R)
            vhx_t = kv.tile([128, NST, HPC, DK + 1], F32R)

            # ---- v projection (vhx) ----
            wv_t = strm.tile([128, NDT, CPG], BF16, tag="wkv")
            for c2 in range(2):
                nc.sync.dma_start(out=wv_t[:, 4 * c2:4 * c2 + 4, :],
                                  in_=wv_r[:, 4 * c2:4 * c2 + 4, :])
            xv_t = strm.tile([128, NDT, S], BF16, tag="xstream")
            for c4 in range(4):
                nc.sync.dma_start(out=xv_t[:, 2 * c4:2 * c4 + 2, :],
                                  in_=xv_r[:, 2 * c4:2 * c4 + 2, :])
            for st in range(NST):
                pv = ps_pv.tile([128, CPG], F32, tag="pv")
                for d in range(NDT):
                    nc.tensor.matmul(pv[:],
                                     xv_t[:, d, st * 128:(st + 1) * 128],
                                     wv_t[:, d, :],
                                     start=(d == 0), stop=(d == NDT - 1))
                nc.vector.tensor_tensor(
                    vhx_t[:, st, :, 0:DK],
                    pv.rearrange("p (h d) -> p h d", h=HPC),
                    bvB_t.rearrange("p (h d) -> p h d", h=HPC),
                    ADD)
                nc.vector.tensor_copy(vhx_t[:, st, :, DK:DK + 1],
                                      ones_t[:].unsqueeze(-1))

            # ---- q projection for block 0 (overlaps with k loads) ----
            wq_t = wts.tile([128, NDT, CPG], BF16)
            for c2 in range(2):
                nc.sync.dma_start(out=wq_t[:, 4 * c2:4 * c2 + 4, :],
                                  in_=wq_r[:, 4 * c2:4 * c2 + 4, :])

            def q_proj(blk):
                sq0 = blk * SQB
                xq_t = wts.tile([128, NDT, SQB], BF16, tag="xq")
                nc.sync.dma_start(out=xq_t[:], in_=xq_r[:, :, sq0:sq0 + SQB])
                qT_t = pB2.tile([128, NCT, SQB], F32R, tag="qT")
                for t in range(NCT):
                    pq = ps_mm.tile([128, SQB], F32, tag="mm")
                    for d in range(NDT):
                        nc.tensor.matmul(pq[:],
                                         wq_t[:, d, t * 128:(t + 1) * 128],
                                         xq_t[:, d, :],
                                         start=(d == 0), stop=(d == NDT - 1))
                    nc.vector.tensor_tensor(
                        qT_t[:, t, :], pq[:],
                        bqk_t[:, t:t + 1].to_broadcast((128, SQB)), ADD)
                return qT_t

            qT_blk = q_proj(0)

            # ---- k projection (kT) ----
            wk_t = strm.tile([128, NDT, CPG], BF16, tag="wkv")
            for c2 in range(2):
                nc.sync.dma_start(out=wk_t[:, 4 * c2:4 * c2 + 4, :],
                                  in_=wk_r[:, 4 * c2:4 * c2 + 4, :])
            xk_t = strm.tile([128, NDT, S], BF16, tag="xstream")
            for c4 in range(4):
                nc.sync.dma_start(out=xk_t[:, 2 * c4:2 * c4 + 2, :],
                                  in_=xk_r[:, 2 * c4:2 * c4 + 2, :])
            for t in range(NCT):
                for blk in range(NB):
                    pk = ps_pv.tile([128, SQB], F32, tag="pv")
                    for d in range(NDT):
                        nc.tensor.matmul(
                            pk[:],
                            wk_t[:, d, t * 128:(t + 1) * 128],
                            xk_t[:, d, blk * SQB:(blk + 1) * SQB],
                            start=(d == 0), stop=(d == NDT - 1))
                    nc.vector.tensor_tensor(
                        kT_t[:, t, blk * SQB:(blk + 1) * SQB], pk[:],
                        bqk_t[:, 4 + t:5 + t].to_broadcast((128, SQB)),
                        ADD)

            # remaining weights while k projection runs
            maskw_t = wts.tile([128, 4, 1024], BF16)
            nc.sync.dma_start(out=maskw_t[:], in_=maskw[:])
            wo_t = wts.tile([128, NCT, D], F32R)
            for c2 in range(2):
                nc.sync.dma_start(out=wo_t[:, 2 * c2:2 * c2 + 2, :],
                                  in_=wo_r[:, 2 * c2:2 * c2 + 2, :])

            def final_proj(outTn_t, sq0, tail=False):
                for dt_i in range(8):
                    if tail:
                        pf = ps_sc.tile([128, SQB], F32, tag="sc")
                    else:
                        pf = ps_mm.tile([128, SQB], F32, tag="mm")
                    for t in range(NCT):
                        nc.tensor.matmul(
                            pf[:],
                            wo_t[:, t, dt_i * 128:(dt_i + 1) * 128],
                            outTn_t[:, t, :],
                            start=(t == 0), stop=(t == NCT - 1))
                    sf = pB2.tile([128, SQB], F32, tag="sf")
                    nc.vector.tensor_copy(sf[:], pf[:])
                    nc.sync.dma_start(
                        out=o[dt_i * 128:(dt_i + 1) * 128, sq0:sq0 + SQB],
                        in_=sf[:])

            pending = []

            # ---- per sq-block attention ----
            for blk in range(NB):
                sq0 = blk * SQB
                nsk = 4 * (blk + 1)
                qT_t = qT_blk
                outTn_t = pB2.tile([128, NCT, SQB], F32R, tag="outTn")
                for t in range(NCT):  # head pair (2t, 2t+1)
                    pvA = ps_pv.tile([65, SQB], F32, tag="pv")
                    pvB = ps_pv.tile([65, SQB], F32, tag="pv")
                    for j in range(nsk):
                        sk0 = j * 128
                        pg = ps_sc.tile([128, 1024], F32, tag="sc")
                        nc.tensor.matmul(pg[:, 0:SQB],
                                         kT_t[0:64, t, sk0:sk0 + 128],
                                         qT_t[0:64, t, :],
                                         start=True, stop=True)
                        nc.tensor.matmul(pg[:, SQB:2 * SQB],
                                         kT_t[64:128, t, sk0:sk0 + 128],
                                         qT_t[64:128, t, :],
                                         start=True, stop=True)
                        eg = pB3.tile([128, 1024], F32R, tag="eg")
                        nc.scalar.activation(eg[:], pg[:], AF.Exp,
                                             bias=0.0, scale=0.125)
                        if j >= blk * 4:
                            jo = j - blk * 4
                            nc.vector.tensor_tensor(
                                eg[:], eg[:], maskw_t[:, jo, :], MUL)
                        nc.tensor.matmul(pvA[:], vhx_t[:, j, 2 * t, :],
                                         eg[:, 0:SQB],
                                         start=(j == 0),
                                         stop=(j == nsk - 1))
                        nc.tensor.matmul(pvB[:], vhx_t[:, j, 2 * t + 1, :],
                                         eg[:, SQB:2 * SQB],
                                         start=(j == 0),
                                         stop=(j == nsk - 1))
                    for hp, pvh in ((0, pvA), (1, pvB)):
                        psl = slice(hp * 64, (hp + 1) * 64)
                        recip = pB2.tile([1, SQB], F32, tag="recip")
                        nc.vector.reciprocal(recip[:], pvh[64:65, :])
                        recipB = pB2.tile([64, SQB], F32, tag="recipB")
                        nc.gpsimd.partition_broadcast(recipB[:], recip[:])
                        nc.vector.tensor_tensor(outTn_t[psl, t, :],
                                                pvh[0:64, :], recipB[:],
                                                MUL)

                if blk + 1 < NB:
                    qT_blk = q_proj(blk + 1)
                pending.append((outTn_t, sq0))
                if blk >= 1:
                    final_proj(*pending.pop(0))
            while pending:
                final_proj(*pending.pop(0), tail=True)

    nc.compile()
    return nc


def _get_program():
    global _PROGRAM
    if _PROGRAM is None:
        _PROGRAM = build_program()
    return _PROGRAM


def _make_maskw():
    import ml_dtypes
    p = np.arange(128, dtype=np.int64)[:, None]
    f = np.arange(512, dtype=np.int64)[None, :]
    mw = np.empty((128, 4, 1024), np.float32)
    for jo in range(4):
        m = (p <= (f - jo * 128)).astype(np.float32)
        mw[:, jo, 0:512] = m
        mw[:, jo, 512:1024] = m
    return mw.astype(ml_dtypes.bfloat16)


def make_in_maps(q, k, v, Wq, bq, Wk, bk, Wv, bv, Wo):
    import ml_dtypes
    BF = ml_dtypes.bfloat16
    wqT, wkT, wvT, woT = Wq.T, Wk.T, Wv.T, Wo.T
    mw = _make_maskw()
    xqT = [np.ascontiguousarray(q[b].T.astype(BF)) for b in range(B)]
    xkT = [np.ascontiguousarray(k[b].T.astype(BF)) for b in range(B)]
    xvT = [np.ascontiguousarray(v[b].T.astype(BF)) for b in range(B)]
    in_maps = []
    for core in range(NCORES):
        b, g = core // 2, core % 2
        cs = slice(g * CPG, (g + 1) * CPG)
        bqk_host = np.empty((128, 8), np.float32)
        for t in range(NCT):
            bqk_host[:, t] = bq[g * CPG + t * 128:g * CPG + (t + 1) * 128]
            bqk_host[:, 4 + t] = bk[g * CPG + t * 128:g * CPG + (t + 1) * 128]
        in_maps.append(dict(
            xq=xqT[b], xk=xkT[b], xv=xvT[b],
            wq=np.ascontiguousarray(wqT[:, cs].astype(BF)),
            wk=np.ascontiguousarray(wkT[:, cs].astype(BF)),
            wv=np.ascontiguousarray(wvT[:, cs].astype(BF)),
            wo=np.ascontiguousarray(woT[cs, :]),
            bqk=bqk_host,
            bv1=np.ascontiguousarray(bv[cs]).reshape(1, CPG),
            maskw=mw,
            ones8=np.ones((128, HPC), np.float32),
        ))
    return in_maps


def assemble_output(results, bo):
    out = np.empty((B, S, D), np.float32)
    for b in range(B):
        acc = results[2 * b]["o"] + results[2 * b + 1]["o"]  # [D, S]
        out[b] = acc.T + bo[None, :]
    return out


def _numpy_fallback(q, k, v, mask, Wq, bq, Wk, bk, Wv, bv, Wo, bo):
    def split_heads(x):
        return x.reshape(B, S, H, DK).transpose(0, 2, 1, 3)

    qh = split_heads(q @ Wq.T + bq)
    kh = split_heads(k @ Wk.T + bk)
    vh = split_heads(v @ Wv.T + bv)
    out = np.empty((B, H, S, DK), np.float32)
    m = np.broadcast_to(np.asarray(mask).reshape(-1, S, S)[-1], (S, S))
    for b in range(B):
        for h in range(H):
            s = (qh[b, h] @ kh[b, h].T) / np.float32(np.sqrt(DK))
            s = np.where(m == 0, np.float32(-1e9), s)
            s = s - s.max(axis=-1, keepdims=True)
            e = np.exp(s)
            a = e / e.sum(axis=-1, keepdims=True)
            out[b, h] = a @ vh[b, h]
    out = out.transpose(0, 2, 1, 3).reshape(B, S, D)
    return out @ Wo.T + bo


def kernel(q, k, v, mask, Wq, bq, Wk, bk, Wv, bv, Wo, bo):
    from concourse.bass_utils import run_bass_kernel_spmd

    q = np.ascontiguousarray(np.asarray(q), dtype=np.float32)
    k = np.ascontiguousarray(np.asarray(k), dtype=np.float32)
    v = np.ascontiguousarray(np.asarray(v), dtype=np.float32)
    Wq, Wk, Wv, Wo = (np.asarray(w, dtype=np.float32) for w in (Wq, Wk, Wv, Wo))
    bq, bk_, bv_, bo = (np.asarray(x, dtype=np.float32) for x in (bq, bk, bv, bo))

    mask_2d = np.asarray(mask).reshape(S, S)
    causal = bool(np.array_equal(mask_2d != 0, np.tril(np.ones((S, S), bool))))
    if not causal:
        return _numpy_fallback(q, k, v, mask, Wq, bq, Wk, bk_, Wv, bv_, Wo, bo)

    nc = _get_program()
    in_maps = make_in_maps(q, k, v, Wq, bq, Wk, bk_, Wv, bv_, Wo)
    res = run_bass_kernel_spmd(nc, in_maps, list(range(NCORES))).results
    return assemble_output(res, bo)


if __name__ == "__main__":
    nc = build_program()
    print("program built + compiled OK")


# revision 26
# speedup vs baseline: 1.1212x; 1.0134x over previous
"""Multi-head causal attention (B=4, S=2048, D=1024, H=16) on 8 trn2 NeuronCores.

Sharding: core = (batch b, head-group g) with b in 0..3, g in 0..1.
Each core computes heads g*8..g*8+7 of batch b end-to-end (QKV projection,
causal softmax attention, and its partial slice of the output projection).
Host sums the two partial output projections per batch and adds bo.

On-device layout: activations are kept "transposed" ([channels, seq] with
channels on SBUF partitions) so every matmul streams a 512-wide free dim at
full fp32r rate:
  qT/kT:  [c, s]   c = head*64+dk channels of this core's 8 heads
  scoresT:[sk, sq] per head; head pairs (2t, 2t+1) are packed into row
          groups 0:64 / 64:128 of the PE array and adjacent PSUM banks
  vhx:    [sk, (h, 65)]  v-heads in natural [s, c] layout with a ones column
          appended per head, so attn@v also produces the softmax denominator
          (row 64 of the PSUM output) for free.
  outT:   [c, sq] normalized by 1/denominator, then Wo^T projection -> o[dout, s]

Projection matmuls (Q/K/V) run in bf16 (halves the input DMA bytes); all
attention matmuls (scores, attn@v, Wo) run in float32r (TF32-like,
~1.5e-4 matmul rel err, full PE rate at free dim >= 256). DRAM inputs are
declared in the matmul dtype so plain HWDGE DMAs feed the PE directly.
Measured end-to-end max rel err vs the fp32 reference: ~2e-3.
"""

import sys

sys.path.insert(0, "/opt/trn_rl_repo")

import numpy as np

B, S, D, H, DK = 4, 2048, 1024, 16, 64
NCORES = 8
CPG = 512          # channels per core (8 heads)
HPC = 8            # heads per core
NB = 4             # sq blocks of 512
SQB = 512
NDT = D // 128     # 8 d-tiles
NCT = CPG // 128   # 4 c-tiles per core (= head pairs)
NST = S // 128     # 16 s-tiles

_PROGRAM = None


def build_program():
    import concourse.tile as tile
    from concourse import mybir, bacc

    F32 = mybir.dt.float32
    F32R = mybir.dt.float32r
    BF16 = mybir.dt.bfloat16
    AF = mybir.ActivationFunctionType
    ADD = mybir.AluOpType.add
    MUL = mybir.AluOpType.mult

    nc = bacc.Bacc("TRN2", target_bir_lowering=False, debug=False,
                   num_devices=NCORES)

    xq = nc.dram_tensor("xq", [D, S], BF16, kind="ExternalInput").ap()
    xk = nc.dram_tensor("xk", [D, S], BF16, kind="ExternalInput").ap()
    xv = nc.dram_tensor("xv", [D, S], BF16, kind="ExternalInput").ap()
    wq = nc.dram_tensor("wq", [D, CPG], BF16, kind="ExternalInput").ap()
    wk = nc.dram_tensor("wk", [D, CPG], BF16, kind="ExternalInput").ap()
    wv = nc.dram_tensor("wv", [D, CPG], BF16, kind="ExternalInput").ap()
    wo = nc.dram_tensor("wo", [CPG, D], F32R, kind="ExternalInput").ap()
    bqk = nc.dram_tensor("bqk", [128, 8], F32, kind="ExternalInput").ap()
    bv1 = nc.dram_tensor("bv1", [1, CPG], F32, kind="ExternalInput").ap()
    maskw = nc.dram_tensor("maskw", [128, 4, 1024], BF16,
                           kind="ExternalInput").ap()
    ones8 = nc.dram_tensor("ones8", [128, 8], F32R, kind="ExternalInput").ap()
    o = nc.dram_tensor("o", [D, S], F32, kind="ExternalOutput").ap()

    xq_r = xq.rearrange("(o p) s -> p o s", p=128)
    xk_r = xk.rearrange("(o p) s -> p o s", p=128)
    xv_r = xv.rearrange("(o p) s -> p o s", p=128)
    wq_r = wq.rearrange("(o p) c -> p o c", p=128)
    wk_r = wk.rearrange("(o p) c -> p o c", p=128)
    wv_r = wv.rearrange("(o p) c -> p o c", p=128)
    wo_r = wo.rearrange("(o p) c -> p o c", p=128)

    with tile.TileContext(nc) as tc:
        with (
            tc.tile_pool(name="wts", bufs=1) as wts,
            tc.tile_pool(name="kv", bufs=1) as kv,
            tc.tile_pool(name="stream", bufs=1) as strm,
            tc.tile_pool(name="phB2", bufs=2) as pB2,
            tc.tile_pool(name="phB3", bufs=3) as pB3,
            tc.tile_pool(name="ps_sc", bufs=2, space="PSUM") as ps_sc,
            tc.tile_pool(name="ps_pv", bufs=3, space="PSUM") as ps_pv,
            tc.tile_pool(name="ps_mm", bufs=1, space="PSUM") as ps_mm,
        ):
            # small constants first (cheap on the DMA queue)
            bqk_t = wts.tile([128, 8], F32)
            bv_row = wts.tile([1, CPG], F32)
            bvB_t = wts.tile([128, CPG], F32)
            ones_t = wts.tile([128, HPC], F32R)
            nc.sync.dma_start(out=bqk_t[:], in_=bqk[:])
            nc.sync.dma_start(out=bv_row[:], in_=bv1[:])
            nc.sync.dma_start(out=ones_t[:], in_=ones8[:])
            nc.gpsimd.partition_broadcast(bvB_t[:], bv_row[:])

            kT_t = kv.tile([128, NCT, S], F32R)
            vhx_t = kv.tile([128, NST, HPC, DK + 1], F32R)

            # ---- v projection (vhx) ----
            wv_t = strm.tile([128, NDT, CPG], BF16, tag="wkv")
            for c2 in range(2):
                nc.sync.dma_start(out=wv_t[:, 4 * c2:4 * c2 + 4, :],
                                  in_=wv_r[:, 4 * c2:4 * c2 + 4, :])
            xv_t = strm.tile([128, NDT, S], BF16, tag="xstream")
            for c4 in range(4):
                nc.sync.dma_start(out=xv_t[:, 2 * c4:2 * c4 + 2, :],
                                  in_=xv_r[:, 2 * c4:2 * c4 + 2, :])
            for st in range(NST):
                pv = ps_pv.tile([128, CPG], F32, tag="pv")
                for d in range(NDT):
                    nc.tensor.matmul(pv[:],
                                     xv_t[:, d, st * 128:(st + 1) * 128],
                                     wv_t[:, d, :],
                                     start=(d == 0), stop=(d == NDT - 1))
                nc.vector.tensor_tensor(
                    vhx_t[:, st, :, 0:DK],
                    pv.rearrange("p (h d) -> p h d", h=HPC),
                    bvB_t.rearrange("p (h d) -> p h d", h=HPC),
                    ADD)
                nc.vector.tensor_copy(vhx_t[:, st, :, DK:DK + 1],
                                      ones_t[:].unsqueeze(-1))

            # ---- q projection for block 0 (overlaps with k loads) ----
            wq_t = wts.tile([128, NDT, CPG], BF16)
            for c2 in range(2):
                nc.sync.dma_start(out=wq_t[:, 4 * c2:4 * c2 + 4, :],
                                  in_=wq_r[:, 4 * c2:4 * c2 + 4, :])

            def q_proj(blk):
                sq0 = blk * SQB
                xq_t = wts.tile([128, NDT, SQB], BF16, tag="xq")
                nc.sync.dma_start(out=xq_t[:], in_=xq_r[:, :, sq0:sq0 + SQB])
                qT_t = pB2.tile([128, NCT, SQB], F32R, tag="qT")
                for t in range(NCT):
                    pq = ps_mm.tile([128, SQB], F32, tag="mm")
                    for d in range(NDT):
                        nc.tensor.matmul(pq[:],
                                         wq_t[:, d, t * 128:(t + 1) * 128],
                                         xq_t[:, d, :],
                                         start=(d == 0), stop=(d == NDT - 1))
                    nc.vector.tensor_tensor(
                        qT_t[:, t, :], pq[:],
                        bqk_t[:, t:t + 1].to_broadcast((128, SQB)), ADD)
                return qT_t

            qT_blk = q_proj(0)

            # ---- k projection (kT) ----
            wk_t = strm.tile([128, NDT, CPG], BF16, tag="wkv")
            for c2 in range(2):
                nc.sync.dma_start(out=wk_t[:, 4 * c2:4 * c2 + 4, :],
                                  in_=wk_r[:, 4 * c2:4 * c2 + 4, :])
            xk_t = strm.tile([128, NDT, S], BF16, tag="xstream")
            for c4 in range(4):
                nc.sync.dma_start(out=xk_t[:, 2 * c4:2 * c4 + 2, :],
                                  in_=xk_r[:, 2 * c4:2 * c4 + 2, :])
            for t in range(NCT):
                for blk in range(NB):
                    pk = ps_pv.tile([128, SQB], F32, tag="pv")
                    for d in range(NDT):
                        nc.tensor.matmul(
                            pk[:],
                            wk_t[:, d, t * 128:(t + 1) * 128],
                            xk_t[:, d, blk * SQB:(blk + 1) * SQB],
                            start=(d == 0), stop=(d == NDT - 1))
                    nc.vector.tensor_tensor(
                        kT_t[:, t, blk * SQB:(blk + 1) * SQB], pk[:],
                        bqk_t[:, 4 + t:5 + t].to_broadcast((128, SQB)),
                        ADD)

            # remaining weights while k projection runs
            maskw_t = wts.tile([128, 4, 1024], BF16)
            nc.sync.dma_start(out=maskw_t[:], in_=maskw[:])
            wo_t = wts.tile([128, NCT, D], F32R)
            for c2 in range(2):
                nc.sync.dma_start(out=wo_t[:, 2 * c2:2 * c2 + 2, :],
                                  in_=wo_r[:, 2 * c2:2 * c2 + 2, :])

            def final_proj(outTn_t, sq0, tail=False):
                for dt_i in range(8):
                    if tail:
                        pf = ps_sc.tile([128, SQB], F32, tag="sc")
                    else:
                        pf = ps_mm.tile([128, SQB], F32, tag="mm")
                    for t in range(NCT):
                        nc.tensor.matmul(
                            pf[:],
                            wo_t[:, t, dt_i * 128:(dt_i + 1) * 128],
                            outTn_t[:, t, :],
                            start=(t == 0), stop=(t == NCT - 1))
                    sf = pB3.tile([128, SQB], F32, tag="sf")
                    nc.vector.tensor_copy(sf[:], pf[:])
                    nc.sync.dma_start(
                        out=o[dt_i * 128:(dt_i + 1) * 128, sq0:sq0 + SQB],
                        in_=sf[:])

            pending = []

            # ---- per sq-block attention ----
            for blk in range(NB):
                sq0 = blk * SQB
                nsk = 4 * (blk + 1)
                qT_t = qT_blk
                outTn_t = pB2.tile([128, NCT, SQB], F32R, tag="outTn")
                for t in range(NCT):  # head pair (2t, 2t+1)
                    pvA = ps_pv.tile([65, SQB], F32, tag="pv")
                    pvB = ps_pv.tile([65, SQB], F32, tag="pv")
                    for j in range(nsk):
                        sk0 = j * 128
                        pg = ps_sc.tile([128, 1024], F32, tag="sc")
                        nc.tensor.matmul(pg[:, 0:SQB],
                                         kT_t[0:64, t, sk0:sk0 + 128],
                                         qT_t[0:64, t, :],
                                         start=True, stop=True)
                        nc.tensor.matmul(pg[:, SQB:2 * SQB],
                                         kT_t[64:128, t, sk0:sk0 + 128],
                                         qT_t[64:128, t, :],
                                         start=True, stop=True)
                        eg = pB3.tile([128, 1024], F32R, tag="eg")
                        nc.scalar.activation(eg[:], pg[:], AF.Exp,
                                             bias=0.0, scale=0.125)
                        if j >= blk * 4:
                            jo = j - blk * 4
                            nc.vector.tensor_tensor(
                                eg[:], eg[:], maskw_t[:, jo, :], MUL)
                        nc.tensor.matmul(pvA[:], vhx_t[:, j, 2 * t, :],
                                         eg[:, 0:SQB],
                                         start=(j == 0),
                                         stop=(j == nsk - 1))
                        nc.tensor.matmul(pvB[:], vhx_t[:, j, 2 * t + 1, :],
                                         eg[:, SQB:2 * SQB],
                                         start=(j == 0),
                                         stop=(j == nsk - 1))
                    for hp, pvh in ((0, pvA), (1, pvB)):
                        psl = slice(hp * 64, (hp + 1) * 64)
                        recip = pB2.tile([1, SQB], F32, tag="recip")
                        nc.vector.reciprocal(recip[:], pvh[64:65, :])
                        recipB = pB2.tile([64, SQB], F32, tag="recipB")
                        nc.gpsimd.partition_broadcast(recipB[:], recip[:])
                        nc.vector.tensor_tensor(outTn_t[psl, t, :],
                                                pvh[0:64, :], recipB[:],
                                                MUL)

                if blk + 1 < NB:
                    qT_blk = q_proj(blk + 1)
                pending.append((outTn_t, sq0))
                if blk >= 1:
                    final_proj(*pending.pop(0))
            while pending:
                final_proj(*pending.pop(0), tail=True)

    nc.compile()
    return nc


def _get_program():
    global _PROGRAM
    if _PROGRAM is None:
        _PROGRAM = build_program()
    return _PROGRAM


def _make_maskw():
    import ml_dtypes
    p = np.arange(128, dtype=np.int64)[:, None]
    f = np.arange(512, dtype=np.int64)[None, :]
    mw = np.empty((128, 4, 1024), np.float32)
    for jo in range(4):
        m = (p <= (f - jo * 128)).astype(np.float32)
        mw[:, jo, 0:512] = m
        mw[:, jo, 512:1024] = m
    return mw.astype(ml_dtypes.bfloat16)


def make_in_maps(q, k, v, Wq, bq, Wk, bk, Wv, bv, Wo):
    import ml_dtypes
    BF = ml_dtypes.bfloat16
    wqT, wkT, wvT, woT = Wq.T, Wk.T, Wv.T, Wo.T
    mw = _make_maskw()
    xqT = [np.ascontiguousarray(q[b].T.astype(BF)) for b in range(B)]
    xkT = [np.ascontiguousarray(k[b].T.astype(BF)) for b in range(B)]
    xvT = [np.ascontiguousarray(v[b].T.astype(BF)) for b in range(B)]
    in_maps = []
    for core in range(NCORES):
        b, g = core // 2, core % 2
        cs = slice(g * CPG, (g + 1) * CPG)
        bqk_host = np.empty((128, 8), np.float32)
        for t in range(NCT):
            bqk_host[:, t] = bq[g * CPG + t * 128:g * CPG + (t + 1) * 128]
            bqk_host[:, 4 + t] = bk[g * CPG + t * 128:g * CPG + (t + 1) * 128]
        in_maps.append(dict(
            xq=xqT[b], xk=xkT[b], xv=xvT[b],
            wq=np.ascontiguousarray(wqT[:, cs].astype(BF)),
            wk=np.ascontiguousarray(wkT[:, cs].astype(BF)),
            wv=np.ascontiguousarray(wvT[:, cs].astype(BF)),
            wo=np.ascontiguousarray(woT[cs, :]),
            bqk=bqk_host,
            bv1=np.ascontiguousarray(bv[cs]).reshape(1, CPG),
            maskw=mw,
            ones8=np.ones((128, HPC), np.float32),
        ))
    return in_maps


def assemble_output(results, bo):
    out = np.empty((B, S, D), np.float32)
    for b in range(B):
        acc = results[2 * b]["o"] + results[2 * b + 1]["o"]  # [D, S]
        out[b] = acc.T + bo[None, :]
    return out


def _numpy_fallback(q, k, v, mask, Wq, bq, Wk, bk, Wv, bv, Wo, bo):
    def split_heads(x):
        return x.reshape(B, S, H, DK).transpose(0, 2, 1, 3)

    qh = split_heads(q @ Wq.T + bq)
    kh = split_heads(k @ Wk.T + bk)
    vh = split_heads(v @ Wv.T + bv)
    out = np.empty((B, H, S, DK), np.float32)
    m = np.broadcast_to(np.asarray(mask).reshape(-1, S, S)[-1], (S, S))
    for b in range(B):
        for h in range(H):
            s = (qh[b, h] @ kh[b, h].T) / np.float32(np.sqrt(DK))
            s = np.where(m == 0, np.float32(-1e9), s)
            s = s - s.max(axis=-1, keepdims=True)
            e = np.exp(s)
            a = e / e.sum(axis=-1, keepdims=True)
            out[b, h] = a @ vh[b, h]
    out = out.transpose(0, 2, 1, 3).reshape(B, S, D)
    return out @ Wo.T + bo


def kernel(q, k, v, mask, Wq, bq, Wk, bk, Wv, bv, Wo, bo):
    from concourse.bass_utils import run_bass_kernel_spmd

    q = np.ascontiguousarray(np.asarray(q), dtype=np.float32)
    k = np.ascontiguousarray(np.asarray(k), dtype=np.float32)
    v = np.ascontiguousarray(np.asarray(v), dtype=np.float32)
    Wq, Wk, Wv, Wo = (np.asarray(w, dtype=np.float32) for w in (Wq, Wk, Wv, Wo))
    bq, bk_, bv_, bo = (np.asarray(x, dtype=np.float32) for x in (bq, bk, bv, bo))

    mask_2d = np.asarray(mask).reshape(S, S)
    causal = bool(np.array_equal(mask_2d != 0, np.tril(np.ones((S, S), bool))))
    if not causal:
        return _numpy_fallback(q, k, v, mask, Wq, bq, Wk, bk_, Wv, bv_, Wo, bo)

    nc = _get_program()
    in_maps = make_in_maps(q, k, v, Wq, bq, Wk, bk_, Wv, bv_, Wo)
    res = run_bass_kernel_spmd(nc, in_maps, list(range(NCORES))).results
    return assemble_output(res, bo)


if __name__ == "__main__":
    nc = build_program()
    print("program built + compiled OK")
